# revision 33
# baseline (speedup 1.0000x reference)
"""Multi-head attention block (QKV proj + SDPA + merge-scramble + fc +
residual + LayerNorm) on 8 Trainium2 NeuronCores.

Sharding: data-parallel over the flattened batch dim (b*n = 32 sequences),
4 sequences per core. Each core runs an identical Bass program on its shard.

Per-sequence math (t = d = e = 512, H = 8 heads, dk = dv = 64):
  Q = qf @ w_q.T ; K = kf @ w_k.T ; V = vf @ w_v.T
  S_h = (Q_h K_h^T) / 8 ;  A_h = softmax(S_h) ;  O_h = A_h V_h
  x = merge_heads(O)            # [t, e]
  x = x.T (the reference's transpose+view scramble; legal since t == e)
  y = LN(x @ w_fc.T + qf) * gamma + beta

On-chip layout strategy: compute Q^T/K^T ([e, t], head-major on partitions)
and V ([t, e]) so that S^T = K_h Q_h^T comes out with tk on partitions.
Softmax then needs no max-subtraction (|S/8| < ~7) and no partition
reduction: exp runs elementwise on ScalarE, and the softmax denominators
fall out of the A^T V matmul by augmenting V with a ones column. The AV
matmul produces O^T tiles which assemble into x^T directly; one PE-transpose
pass converts x^T -> x for the fc matmul (the scramble means fc contracts
over the *time* index, so x must have time on partitions).

Matmuls run in float32r (TF32-ish split mode, 1 cycle/row at N>=512,
~1.5e-4 rel err) with fp32 PSUM accumulation.
"""

import contextlib

import numpy as np

import concourse.bacc as bacc
import concourse.mybir as mybir
import concourse.tile as tile
from concourse.bass_utils import run_bass_kernel_spmd
from concourse.masks import make_identity


@contextlib.contextmanager
def _one_act_table_set():
    """Steer the act-table placement pass to natural_log_exp_and_others for
    both Exp and Ln (it otherwise maps each function to the first set that
    contains it, thrashing exp_and_others <-> natural_log every sequence; a
    reload is ~2.7us on HW). Entries are blanked, not removed, so the
    act_func_set_id indices still line up with act_info.json."""
    orig = bacc.get_activation_tables

    def patched(arch):
        tabs = dict(orig(arch))
        for name in tabs:
            if name != "natural_log_exp_and_others":
                tabs[name] = set()
        return tabs

    bacc.get_activation_tables = patched
    try:
        yield
    finally:
        bacc.get_activation_tables = orig

F32 = mybir.dt.float32
F32R = mybir.dt.float32r
BF16 = mybir.dt.bfloat16
AF = mybir.ActivationFunctionType
OP = mybir.AluOpType

N_CORES = 8
S = 4          # sequences per core
T = 512        # sequence length
D = 512        # model dim (= e = n_head * d_k)
NH = 8         # heads
DV = 64        # head dim
C = 4          # 128-row chunks per 512 dim
P = 128
EPS = 1e-6

_PROGRAM_CACHE = {}

# production configuration (the `ablate` mechanism doubles as a config knob
# set for timing experiments; these two are numerically-safe config choices)
PROD_FLAGS = frozenset({"sums_dve", "bf16"})


def _build_program(apply_affine: bool, loop_iters: int = 1,
                   ablate: frozenset = frozenset(), emits_per_iter: int = 1):
    # `ablate` drops instruction classes for timing experiments only (the
    # result is garbage): {"exp", "indma", "copies", "mm", "scoremm"}
    ab = ablate
    UNI = "uni8" in ab
    DT = BF16 if "bf16" in ab else F32R
    nc = bacc.Bacc()

    qT = nc.declare_dram_parameter("qT", [S, D, T], DT, isOutput=False)
    kT = nc.declare_dram_parameter("kT", [S, D, T], DT, isOutput=False)
    vT = nc.declare_dram_parameter("vT", [S, D, T], DT, isOutput=False)
    qn = nc.declare_dram_parameter(
        "qn", [S, T, D], DT if "dmafew" in ab else F32, isOutput=False)
    wq = nc.declare_dram_parameter("wq", [D, D], DT, isOutput=False)  # w_q.T
    wk = nc.declare_dram_parameter("wk", [D, D], DT, isOutput=False)  # w_k.T
    wv = nc.declare_dram_parameter("wv", [D, D], DT, isOutput=False)  # w_v.T
    wfc = nc.declare_dram_parameter("wfc", [D, D], DT, isOutput=False)  # w_fc.T
    if apply_affine:
        gmb = nc.declare_dram_parameter("gmb", [P, D], F32, isOutput=False)
        btb = nc.declare_dram_parameter("btb", [P, D], F32, isOutput=False)
    out = nc.declare_dram_parameter(
        "out", [S, T, D], DT if "dmafew" in ab else F32, isOutput=True)

    with tile.TileContext(nc) as tc:
        with (
            tc.tile_pool(name="const", bufs=1) as cst,
            tc.tile_pool(name="inp", bufs=3 if "deep" in ablate else 2) as inp,
            tc.tile_pool(name="proj", bufs=3 if "deep" in ablate else 2) as proj,
            tc.tile_pool(name="expp", bufs=8 if "deep" in ablate else 6) as expp,
            tc.tile_pool(name="xp", bufs=3 if "deep" in ablate else 2) as xp,
            tc.tile_pool(name="small", bufs=2) as small,
            tc.tile_pool(name="psc", bufs=2, space="PSUM") as psc,
            tc.tile_pool(name="pfc", bufs=3 if "ring3" in ab else 2,
                         space="PSUM") as pfc,
            tc.tile_pool(name="pav", bufs=4 if "pav4" in ab else 2,
                         space="PSUM") as pavp,
            tc.tile_pool(name="pu", bufs=8, space="PSUM") as pu,
        ):
            # one-time constants; weight DMAs split per 128-row chunk so the
            # first projection matmuls start as soon as chunk 0 lands.
            wq_sb = cst.tile([P, C, D], DT, tag="wq")
            wk_sb = cst.tile([P, C, D], DT, tag="wk")
            wv_sb = cst.tile([P, C, D], DT, tag="wv")
            wfc_sb = cst.tile([P, C, D], DT, tag="wfc")
            ident = cst.tile([P, P], F32, tag="ident")
            make_identity(nc, ident[:])
            identb = cst.tile([P, P], BF16, tag="identb")
            nc.vector.tensor_copy(identb[:], ident[:])
            eps_sb = cst.tile([P, 1], F32, tag="eps")
            nc.vector.memset(eps_sb[:], EPS)
            if apply_affine:
                gm_sb = cst.tile([P, D], F32, tag="gmb")
                bt_sb = cst.tile([P, D], F32, tag="btb")
                nc.sync.dma_start(gm_sb[:], gmb[:])
                nc.sync.dma_start(bt_sb[:], btb[:])

            def load(s, weight_dmas=None):
                st = {}
                st["qT"] = inp.tile([P, C, T], DT, tag="qT", name="qT_sb")
                st["kT"] = inp.tile([P, C, T], DT, tag="kT", name="kT_sb")
                st["vT"] = inp.tile([P, C, T], DT, tag="vT", name="vT_sb")
                # consumption order: (wq,qT) all chunks, then (wk,kT), (wv,vT)
                for (sb, dr), w_pair in zip(
                    ((st["qT"], qT), (st["kT"], kT), (st["vT"], vT)),
                    weight_dmas or ((), (), ()),
                ):
                    if ("dmabig" in ab or "dmafew" in ab) and "indma" not in ab:
                        for w_sb, w in w_pair:
                            nc.sync.dma_start(
                                w_sb[:], w.rearrange("(c p) e -> p c e", p=P)
                            )
                        nc.sync.dma_start(
                            sb[:], dr[s].rearrange("(c p) t -> p c t", p=P)
                        )
                        continue
                    for dc in range(C):
                        for w_sb, w in w_pair:
                            nc.sync.dma_start(
                                w_sb[:, dc, :],
                                w.rearrange("(c p) e -> p c e", p=P)[:, dc, :],
                            )
                        if "indma" not in ab:
                            nc.sync.dma_start(
                                sb[:, dc, :],
                                dr[s].rearrange("(c p) t -> p c t", p=P)[:, dc, :],
                            )
                        else:
                            nc.vector.memset(sb[:, dc, 0:1], 0.5)
                return st

            def projA(s, st):
                # Q^T/K^T [e, t] head-major; V [t, e] with per-head ones col
                st["QT"] = proj.tile([P, C, T], DT, tag="QT", name="QT_sb")
                st["KT"] = proj.tile([P, C, T], DT, tag="KT", name="KT_sb")
                for dst, w_sb, x_sb in (
                    (st["QT"], wq_sb, st["qT"]), (st["KT"], wk_sb, st["kT"])
                ):
                    for ec in range(C):
                        ps = (pu.tile([P, T], F32, tag="u", name="ps") if UNI
                              else pfc.tile([P, T], F32, tag="fc", name="ps"))
                        ndc = 1 if "mmlite" in ab else C
                        for dc in range(ndc):
                            nc.tensor.matmul(
                                ps[:],
                                lhsT=w_sb[:, dc, ec * P:(ec + 1) * P],
                                rhs=x_sb[:, dc, :],
                                start=(dc == 0),
                                stop=(dc == ndc - 1),
                            )
                        if "copies" not in ab:
                            nc.vector.tensor_copy(dst[:, ec, :], ps[:])
                        else:
                            nc.vector.tensor_copy(dst[:, ec, 0:1], ps[:, 0:1])
                V_sb = proj.tile([P, C, NH, DV + 1], BF16, tag="V", name="V_sb")
                st["V"] = V_sb
                nc.gpsimd.memset(V_sb[:, :, :, DV:DV + 1], 1.0)
                for tc_ in range(C):
                    ps = (pu.tile([P, T], F32, tag="u", name="ps") if UNI
                          else pfc.tile([P, T], F32, tag="fc", name="ps"))
                    ndc = 1 if "mmlite" in ab else C
                    for dc in range(ndc):
                        nc.tensor.matmul(
                            ps[:],
                            lhsT=st["vT"][:, dc, tc_ * P:(tc_ + 1) * P],
                            rhs=wv_sb[:, dc, :],
                            start=(dc == 0),
                            stop=(dc == ndc - 1),
                        )
                    if "copies" not in ab:
                        vcopy = (nc.vector.tensor_copy if "v_dve" in ab
                                 else nc.scalar.copy)
                        vcopy(
                            V_sb[:, tc_, :, 0:DV],
                            ps.rearrange("p (h v) -> p h v", h=NH),
                        )
                    else:
                        nc.scalar.copy(
                            V_sb[:, tc_, :, 0:1],
                            ps.rearrange("p (h v) -> p h v", h=NH)[:, :, 0:1],
                        )

            def attnB(s, st):
                # S^T = K_h Q_h^T / 8 with tk on partitions -> exp elementwise
                # (no max subtraction; |S/8| <~ 7) -> O^T = V_aug^T A^T, whose
                # ones row yields the softmax denominators for free. Heads are
                # paired: rows 0-63/64-127 of a KT/QT chunk are disjoint PE
                # row groups, so back-to-back K=64 matmuls run concurrently.
                # Score psums are [P, 1024] (two banks) so one exp covers two
                # tk chunks; AV of head-pair hp-1 is emitted between the score
                # pairs of hp so PE fills the exp-wait gaps.
                xT_sb = xp.tile([P, C, T], DT, tag="xT", name="xT_sb")
                sA = xp.tile([P, T], DT, tag="sA", name="sA")
                sB = xp.tile([P, T], DT, tag="sB", name="sB")
                st["sA"], st["sB"] = sA, sB
                st["xT"] = xT_sb
                nc.gpsimd.memset(sA[:], 1.0)
                nc.gpsimd.memset(sB[:], 1.0)

                def scores(hp, pair, expSs):
                    if "ring3" in ab or "pav4" in ab or UNI:
                        for j in range(2):
                            tkc = 2 * pair + j
                            for sub in range(2):
                                ps = (pu.tile([P, T], F32, tag="u", name="ps")
                                      if UNI else
                                      psc.tile([P, T], F32, tag="sc", name="ps"))
                                nc.tensor.matmul(
                                    ps[:],
                                    lhsT=st["KT"][sub * DV:(sub + 1) * DV, hp,
                                                  tkc * P:(tkc + 1) * P],
                                    rhs=st["QT"][sub * DV:(sub + 1) * DV, hp, :],
                                    start=True,
                                    stop=True,
                                )
                                nc.scalar.activation(
                                    expSs[sub][:, tkc, :], ps[:], AF.Exp,
                                    scale=0.125,
                                )
                        return
                    pss = [
                        psc.tile([P, 2 * T], F32, tag="sc", name="ps")
                        for _ in range(2)
                    ]
                    for j in range(2):
                        tkc = 2 * pair + j
                        for sub in range(2):
                            nc.tensor.matmul(
                                pss[sub][:, j * T:(j + 1) * T],
                                lhsT=st["KT"][sub * DV:(sub + 1) * DV, hp,
                                              tkc * P:(tkc + 1) * P],
                                rhs=st["QT"][sub * DV:(sub + 1) * DV, hp, :],
                                start=True,
                                stop=True,
                            )
                    for sub in range(2):
                        if "exp" in ab:
                            nc.scalar.activation(
                                expSs[sub][:, 2 * pair:2 * pair + 2, 0:1],
                                pss[sub].rearrange(
                                    "p (a b) -> p a b", a=2)[:, :, 0:1],
                                AF.Exp,
                                scale=0.125,
                            )
                            continue
                        nc.scalar.activation(
                            expSs[sub][:, 2 * pair:2 * pair + 2, :],
                            pss[sub].rearrange("p (a b) -> p a b", a=2),
                            AF.Exp,
                            scale=0.125,
                        )

                def av_head(hp, sub, expSs):
                    h = 2 * hp + sub
                    pav = (pu.tile([DV + 1, T], F32, tag="u", name="pav")
                           if UNI else
                           pavp.tile([DV + 1, T], F32, tag="av", name="pav"))
                    ntk = 1 if "mmlite" in ab else C
                    for tkc in range(ntk):
                        nc.tensor.matmul(
                            pav[:],
                            lhsT=st["V"][:, tkc, h, :],
                            rhs=expSs[sub][:, tkc, :],
                            start=(tkc == 0),
                            stop=(tkc == ntk - 1),
                        )
                    if "copies" not in ab:
                        nc.vector.tensor_copy(
                            xT_sb[sub * DV:(sub + 1) * DV, hp, :], pav[0:DV, :]
                        )
                        # ScalarE, not DVE: balances engine load (DVE is the
                        # busier of the two) and ScE reads PSUM faster.
                        s_t = sA if h < 4 else sB
                        scopy = (nc.vector.tensor_copy if "sums_dve" in ab
                                 else nc.scalar.copy)
                        scopy(
                            s_t[32 * (h % 4):32 * (h % 4) + 1, :],
                            pav[DV:DV + 1, :],
                        )
                    else:
                        nc.vector.tensor_copy(
                            xT_sb[sub * DV:(sub + 1) * DV, hp, 0:1],
                            pav[0:DV, 0:1],
                        )
                        s_t = sA if h < 4 else sB
                        nc.scalar.copy(
                            s_t[32 * (h % 4):32 * (h % 4) + 1, 0:1],
                            pav[DV:DV + 1, 0:1],
                        )

                pend = []
                for hp in range(NH // 2):
                    expSs = [
                        expp.tile([P, C, T], BF16, tag="expS", name=f"expS{sub}")
                        for sub in range(2)
                    ]
                    scores(hp, 0, expSs)
                    if len(pend) >= 2:
                        av_head(*pend.pop(0))
                    scores(hp, 1, expSs)
                    if len(pend) >= 2:
                        av_head(*pend.pop(0))
                    pend.append((hp, 0, expSs))
                    pend.append((hp, 1, expSs))
                for p_ in pend:
                    av_head(*p_)

            def tailC(s, st):
                # prefetch the residual rows early
                if "dmafew" in ab:
                    qn_sb = small.tile([P, C, D], DT, tag="qn", bufs=2,
                                       name="qn_sb")
                    if "indma" not in ab:
                        nc.sync.dma_start(
                            qn_sb[:], qn[s].rearrange("(c p) d -> p c d", p=P))
                    else:
                        nc.gpsimd.memset(qn_sb[:, 0, 0:1], 0.5)
                    qn_cs = [qn_sb[:, ac, :] for ac in range(C)]
                else:
                    qn_cs = []
                    for ac in range(C):
                        qn_c = small.tile([P, D], F32, tag="qn", bufs=4,
                                          name="qn_c")
                        if "indma" not in ab:
                            dma_in = (nc.gpsimd.dma_start if "dmaq" in ab
                                      else nc.sync.dma_start)
                            dma_in(qn_c[:], qn[s, ac * P:(ac + 1) * P, :])
                        else:
                            nc.gpsimd.memset(qn_c[:, 0:1], 0.5)
                        qn_cs.append(qn_c)
                st2_seq = small.tile([P, C, 2], F32, tag="st2", name="st2_seq")
                y_cs = []

                # R = 1/softmax-sums transposed to [tq, head]: sA/sB rows
                # {0,32,64,96} hold the sums; PE-transpose 128-col blocks and
                # take reciprocals of columns {0,32,64,96}.
                R_sb = small.tile([P, C, NH], F32, tag="R", name="R_sb")
                for c in range(C):
                    trS = (pu.tile([P, T], F32, tag="u", name="trS") if UNI
                           else pavp.tile([P, T], DT, tag="av", name="trS")
                           if DT is BF16 else
                           pfc.tile([P, T], F32, tag="fc", name="trS"))
                    idt = identb if trS.dtype == BF16 else ident
                    for i, s_t in enumerate((st["sA"], st["sB"])):
                        nc.tensor.transpose(
                            trS[:, i * P:(i + 1) * P],
                            s_t[:, c * P:(c + 1) * P],
                            idt[:],
                        )
                    nc.vector.reciprocal(R_sb[:, c, 0:4], trS[:, 0:97:32])
                    nc.vector.reciprocal(R_sb[:, c, 4:8], trS[:, P:P + 97:32])

                # x^T -> x (PE transpose) fused with softmax normalization
                x_nat = xp.tile([P, C, T], DT, tag="xnat", name="x_nat")
                for c in range(C):
                    ptr = (pu.tile([P, T], F32, tag="u", name="ptr") if UNI
                           else pavp.tile([P, T], DT, tag="av", name="ptr")
                           if DT is BF16 else
                           pfc.tile([P, T], F32, tag="fc", name="ptr"))
                    idt = identb if ptr.dtype == BF16 else ident
                    for ec in range(C):
                        nc.tensor.transpose(
                            ptr[:, ec * P:(ec + 1) * P],
                            st["xT"][:, ec, c * P:(c + 1) * P],
                            idt[:],
                        )
                    nc.vector.tensor_tensor(
                        x_nat[:, c, :].rearrange("p (h v) -> p h v", h=NH),
                        ptr.rearrange("p (h v) -> p h v", h=NH),
                        R_sb[:, c, :, None].to_broadcast((P, NH, DV)),
                        OP.mult,
                    )

                # fc (contracting over the *time* index, thanks to the
                # reference's transpose-view scramble) + residual + LayerNorm
                y_sb = (small.tile([P, C, D], F32, tag="y", bufs=2,
                                   name="y_sb")
                        if "dmafew" in ab else None)
                for ac in range(C):
                    psy = (pu.tile([P, T], F32, tag="u", name="psy") if UNI
                           else pfc.tile([P, T], F32, tag="fc", name="psy"))
                    ncc = 1 if "mmlite" in ab else C
                    for cc in range(ncc):
                        nc.tensor.matmul(
                            psy[:],
                            lhsT=x_nat[:, cc, ac * P:(ac + 1) * P],
                            rhs=wfc_sb[:, cc, :],
                            start=(cc == 0),
                            stop=(cc == ncc - 1),
                        )
                    y_c = (y_sb[:, ac, :] if "dmafew" in ab else
                           small.tile([P, D], F32, tag="y", bufs=4, name="y_c"))
                    nc.vector.tensor_tensor(y_c[:], psy[:], qn_cs[ac][:], OP.add)
                    st6 = small.tile([P, 6], F32, tag="st6", name="st6")
                    nc.vector.bn_stats(st6[:], y_c[:])
                    nc.vector.bn_aggr(st2_seq[:, ac, :], st6[:])
                    y_cs.append(y_c)
                # rsqrt(var+eps) = exp(-ln(var+eps)/2): Ln and Exp share one
                # ACT table set (natural_log_exp_and_others), so the LN tail
                # never evicts the attention Exp tables (a reload is ~2.7us).
                lg = small.tile([P, C], F32, tag="sd", name="lg")
                rinv = small.tile([P, C], F32, tag="rinv", name="rinv")
                nc.scalar.activation(lg[:], st2_seq[:, :, 1], AF.Ln, bias=eps_sb[:])
                nc.scalar.activation(rinv[:], lg[:], AF.Exp, scale=-0.5)
                yo_sb = (small.tile([P, C, D], DT, tag="yo", bufs=2,
                                    name="yo_sb")
                         if "dmafew" in ab else None)
                for ac in range(C):
                    y_c = y_cs[ac]
                    y_o = yo_sb[:, ac, :] if "dmafew" in ab else y_c
                    nc.vector.tensor_scalar(
                        y_o[:], y_c[:], st2_seq[:, ac, 0:1], rinv[:, ac:ac + 1],
                        OP.subtract, OP.mult,
                    )
                    if apply_affine:
                        nc.vector.tensor_tensor(y_o[:], y_o[:], gm_sb[:], OP.mult)
                        nc.vector.tensor_tensor(y_o[:], y_o[:], bt_sb[:], OP.add)
                    if "dmafew" not in ab:
                        dma_out = (nc.gpsimd.dma_start if "dmaq" in ab
                                   else nc.sync.dma_start)
                        dma_out(out[s, ac * P:(ac + 1) * P, :], y_o[:])
                if "dmafew" in ab:
                    nc.sync.dma_start(
                        out[s].rearrange("(c p) d -> p c d", p=P), yo_sb[:])

            # software-pipelined emission: proj of seq s+1 is emitted before
            # the tail of seq s so the scheduler can fill PE gaps in the
            # attention/normalize phases with next-sequence matmuls.
            def emit_all(first=False):
                sts = {}
                sts[0] = load(0, weight_dmas=(
                    ((wq_sb, wq),),
                    ((wk_sb, wk),),
                    ((wv_sb, wv), (wfc_sb, wfc)),
                ) if first else None)
                projA(0, sts[0])
                sts[1] = load(1)
                attnB(0, sts[0])
                for s in range(1, S):
                    projA(s, sts[s])
                    if s + 1 < S:
                        sts[s + 1] = load(s + 1)
                    tailC(s - 1, sts[s - 1])
                    attnB(s, sts[s])
                tailC(S - 1, sts[S - 1])

            if loop_iters == 1:
                for i in range(emits_per_iter):
                    emit_all(first=(i == 0))
            else:
                # weights are loaded once ahead of the loop (the single-shot
                # kernel also loads them once); the loop body must not re-DMA
                # them or the boundary serializes on the reload.
                for dc in range(C):
                    for w_sb, w in ((wq_sb, wq), (wk_sb, wk), (wv_sb, wv),
                                    (wfc_sb, wfc)):
                        nc.sync.dma_start(
                            w_sb[:, dc, :],
                            w.rearrange("(c p) e -> p c e", p=P)[:, dc, :],
                        )
                with tc.For_i(0, loop_iters, 1):
                    for _ in range(emits_per_iter):
                        emit_all()

    with _one_act_table_set():
        nc.finalize()
    return nc


def _get_program(apply_affine: bool, loop_iters: int = 1,
                 ablate: frozenset = frozenset(), emits_per_iter: int = 1):
    key = (apply_affine, loop_iters, ablate, emits_per_iter)
    if key not in _PROGRAM_CACHE:
        _PROGRAM_CACHE[key] = _build_program(
            apply_affine, loop_iters, ablate, emits_per_iter)
    return _PROGRAM_CACHE[key]


def kernel(q, k, v, w_q, w_k, w_v, w_fc, ln_gamma, ln_beta, _res_holder=None):
    q = np.asarray(q, dtype=np.float32)
    k = np.asarray(k, dtype=np.float32)
    v = np.asarray(v, dtype=np.float32)
    w_q = np.asarray(w_q, dtype=np.float32)
    w_k = np.asarray(w_k, dtype=np.float32)
    w_v = np.asarray(w_v, dtype=np.float32)
    w_fc = np.asarray(w_fc, dtype=np.float32)
    ln_gamma = np.asarray(ln_gamma, dtype=np.float32)
    ln_beta = np.asarray(ln_beta, dtype=np.float32)

    b, n, t, d = q.shape
    B = b * n
    assert (b, n, t, d) == (8, 4, T, D), q.shape
    qf = q.reshape(B, t, d)
    kf = k.reshape(B, t, d)
    vf = v.reshape(B, t, d)

    apply_affine = not (
        np.all(ln_gamma == 1.0) and np.all(ln_beta == 0.0)
    )
    nc = _get_program(apply_affine, ablate=PROD_FLAGS)

    bf = np.dtype(mybir.dt.np(BF16))
    wq_t = np.ascontiguousarray(w_q.T).astype(bf)
    wk_t = np.ascontiguousarray(w_k.T).astype(bf)
    wv_t = np.ascontiguousarray(w_v.T).astype(bf)
    wfc_t = np.ascontiguousarray(w_fc.T).astype(bf)

    in_maps = []
    for c in range(N_CORES):
        sl = slice(S * c, S * (c + 1))
        m = {
            "qT": np.ascontiguousarray(qf[sl].transpose(0, 2, 1)).astype(bf),
            "kT": np.ascontiguousarray(kf[sl].transpose(0, 2, 1)).astype(bf),
            "vT": np.ascontiguousarray(vf[sl].transpose(0, 2, 1)).astype(bf),
            "qn": np.ascontiguousarray(qf[sl]),
            "wq": wq_t, "wk": wk_t, "wv": wv_t, "wfc": wfc_t,
        }
        if apply_affine:
            m["gmb"] = np.ascontiguousarray(
                np.broadcast_to(ln_gamma, (P, D)).astype(np.float32)
            )
            m["btb"] = np.ascontiguousarray(
                np.broadcast_to(ln_beta, (P, D)).astype(np.float32)
            )
        in_maps.append(m)

    res = run_bass_kernel_spmd(nc, in_maps, list(range(N_CORES)))
    if _res_holder is not None:
        _res_holder.append(res)
    full = np.concatenate([res.results[c]["out"] for c in range(N_CORES)], axis=0)
    return full.reshape(b, n, t, d).astype(np.float32)

